# revision 15
# baseline (speedup 1.0000x reference)
"""BiLinearAttention Trainium2 kernel (8-core data-parallel over batch).

reference math (per batch b):
    cw      = context[b] @ W                        # [S, E]
    scores  = cw @ query[:, b, :].T  (as [Q, S])
    scores  = where(mask[b], -1e30, scores)
    attn    = softmax(scores, axis=-1)              # [Q, S]
    comp    = attn @ context[b]                     # [Q, D]
outputs: attn_out [Q, B, S], composition [Q, B, D]

Device mapping (per core, 2 batches):
  The two logit matmuls (context@W and cw@q^T) need ~fp32 precision: the
  softmax logits have std ~1024, so low-precision matmuls perturb the
  attention weights catastrophically on near-tie rows. Native fp32 matmul
  runs at 4 cycles/row on the PE. Instead each fp32 operand x is split as
  x = hi + lo (hi = fp16(x), lo = fp16(x - hi), together ~23 mantissa
  bits) and each logical matmul becomes hi*hi + hi*lo + lo*hi at
  1 cycle/row each: 3/4 the PE time of native fp32 with ~fp32 accuracy.
  W/ctxT/qT are split on the host; cwT is split on-device out of PSUM.

  phase A : cwT[e, s] += W[d, e]^T ctxT[d, s]      (fp16 x3)
  scores  : 24 accumulating fp16 matmuls per [128,512] psum chunk
            (lhsT=qT[e-tile, q-tile], rhs=cwT[e-tile, s-chunk]); the mask
            bias is fused into the psum->SBUF copy as a DVE
            scalar_tensor_tensor add against a once-per-batch broadcast
            tile (built by a ones[1,128] (x) bias_row matmul). bias =
            -60000 stands in for the reference's -1e30: fp16-safe, and
            exp(masked - max) underflows to exactly 0 either way.
  softmax : DVE max (negated) -> ACT exp(x - max) with accumulated row
            sum -> DVE reciprocal -> DVE in-place scale (all fp32; the
            fp32 attn tile is DMA'd out directly)
  comp    : attn is also rounded to fp16; PE transposes its 128x128
            tiles (fp16, 1 cyc/row) and 16 fp16 matmuls vs host-rounded
            fp16 context accumulate each [q,d] block. fp16 here is
            harmless: attn is near-one-hot in [0,1] and context is O(1),
            so composition carries ~1e-3 relative error.
"""

import sys

if "/opt/trn_rl_repo" not in sys.path:
    sys.path.insert(0, "/opt/trn_rl_repo")

import numpy as np

B, S, Q, D, E = 16, 2048, 2048, 1024, 1024
NCORES = 8
BLOC = B // NCORES  # batches per core
P = 128
SA = 512   # phase-A s-chunk width
SCH = 512  # scores / comp free-dim chunk width
MASK_BIAS = np.float16(-60000.0)

LAST_RESULTS = None  # BassKernelResults of the most recent run (for test harness)

_PROG = None


def _build_program():
    from concourse import bacc, mybir, tile
    from concourse.masks import make_identity

    f32 = mybir.dt.float32
    f16 = mybir.dt.float16
    AX = mybir.AxisListType
    AF = mybir.ActivationFunctionType

    nc = bacc.Bacc("TRN2", target_bir_lowering=False, debug=False,
                   num_devices=NCORES)
    ctxh_d = nc.dram_tensor("ctxh", [BLOC, S, D], f16, kind="ExternalInput")
    ctxTh_d = nc.dram_tensor("ctxTh", [BLOC, D, S], f16, kind="ExternalInput")
    ctxTl_d = nc.dram_tensor("ctxTl", [BLOC, D, S], f16, kind="ExternalInput")
    qTh_d = nc.dram_tensor("qTh", [BLOC, E, Q], f16, kind="ExternalInput")
    qTl_d = nc.dram_tensor("qTl", [BLOC, E, Q], f16, kind="ExternalInput")
    wh_d = nc.dram_tensor("wh", [D, E], f16, kind="ExternalInput")
    wl_d = nc.dram_tensor("wl", [D, E], f16, kind="ExternalInput")
    bias_d = nc.dram_tensor("biasr", [BLOC, S], f16, kind="ExternalInput")
    attn_d = nc.dram_tensor("attn_out", [Q, BLOC, S], f32, kind="ExternalOutput")
    comp_d = nc.dram_tensor("comp_out", [Q, BLOC, D], f32, kind="ExternalOutput")

    DT, ET, QT, ST = D // P, E // P, Q // P, S // P

    with tile.TileContext(nc) as tc:
        with (
            tc.tile_pool(name="const", bufs=1) as constp,
            tc.tile_pool(name="resident", bufs=1) as resp,
            tc.tile_pool(name="qtp", bufs=2) as qtp,
            tc.tile_pool(name="scoresp", bufs=1) as scoresp,
            tc.tile_pool(name="attnp", bufs=1) as attnp,
            tc.tile_pool(name="attnhp", bufs=1) as attnhp,
            tc.tile_pool(name="attnTp", bufs=4) as attnTp,
            tc.tile_pool(name="ctxT4p", bufs=2) as ctxT4p,
            tc.tile_pool(name="stgp", bufs=1) as stgp,
            tc.tile_pool(name="csbp", bufs=1) as csbp,
            tc.tile_pool(name="smallp", bufs=2) as smallp,
            tc.tile_pool(name="ps_mm", bufs=3, space="PSUM") as ps_mm,
            tc.tile_pool(name="ps_tr", bufs=3, space="PSUM") as ps_tr,
            tc.tile_pool(name="ps_cp", bufs=2, space="PSUM") as ps_cp,
        ):
            wh = constp.tile([P, DT, E], f16)
            wl = constp.tile([P, DT, E], f16)
            wh_r = wh_d.ap().rearrange("(dt dp) e -> dp dt e", dp=P)
            wl_r = wl_d.ap().rearrange("(dt dp) e -> dp dt e", dp=P)
            # chunked so the first phase-A group isn't gated on all 4 MiB
            for ec in range(ET):
                es = slice(ec * P, (ec + 1) * P)
                nc.sync.dma_start(wh[:, :, es], wh_r[:, :, es])
                nc.sync.dma_start(wl[:, :, es], wl_r[:, :, es])
            ones = constp.tile([1, P], f16)
            nc.vector.memset(ones[:], 1.0)
            id_h = constp.tile([P, P], f16)
            id_f32 = stgp.tile([P, P], f32, tag="stg")
            make_identity(nc, id_f32[:])
            nc.vector.tensor_copy(id_h[:], id_f32[:])

            for b in range(BLOC):
                bias_row = resp.tile([1, S], f16, tag="bias_row")
                nc.sync.dma_start(bias_row[:], bias_d.ap()[b:b + 1, :])
                cwh = resp.tile([P, ET, S], f16, tag="cwh")
                cwl = resp.tile([P, ET, S], f16, tag="cwl")
                ctx_h = resp.tile([P, ST, D], f16, tag="ctxh")

                # phase A: cw[e, s] = sum_d W[d, e] * ctxT[d, s], split hi/lo
                ctxTh_r = ctxTh_d.ap()[b].rearrange("(dt dp) s -> dp dt s", dp=P)
                ctxTl_r = ctxTl_d.ap()[b].rearrange("(dt dp) s -> dp dt s", dp=P)
                for sc in range(S // SA):
                    cth = ctxT4p.tile([P, DT, SA], f16, tag="cth")
                    ctl = ctxT4p.tile([P, DT, SA], f16, tag="ctl")
                    nc.sync.dma_start(
                        cth[:], ctxTh_r[:, :, sc * SA:(sc + 1) * SA])
                    nc.sync.dma_start(
                        ctl[:], ctxTl_r[:, :, sc * SA:(sc + 1) * SA])
                    for e in range(ET):
                        for half in range(SA // SCH):
                            ps = ps_mm.tile([P, SCH], f32, tag="psmm")
                            hs = slice(half * SCH, (half + 1) * SCH)
                            # hi*hi and lo*hi first: they gate only on cth,
                            # so the ctl DMA overlaps the first 16 matmuls
                            for d in range(DT):
                                we = (slice(None), d, slice(e * P, (e + 1) * P))
                                nc.tensor.matmul(ps[:], wh[we], cth[:, d, hs],
                                                 start=(d == 0), stop=False)
                                nc.tensor.matmul(ps[:], wl[we], cth[:, d, hs],
                                                 start=False, stop=False)
                            for d in range(DT):
                                we = (slice(None), d, slice(e * P, (e + 1) * P))
                                nc.tensor.matmul(ps[:], wh[we], ctl[:, d, hs],
                                                 start=False, stop=(d == DT - 1))
                            sl = (slice(None), e,
                                  slice(sc * SA + half * SCH,
                                        sc * SA + (half + 1) * SCH))
                            nc.vector.tensor_copy(cwh[sl], ps[:])
                            nc.vector.tensor_sub(cwl[sl], ps[:], cwh[sl])

                # mask-bias broadcast tile, built after phase A so its
                # (bufs=1) slot swap never stalls the batch boundary
                bias_bc = resp.tile([P, S], f32, tag="bias_bc")
                for ch in range(S // SCH):
                    psb = ps_mm.tile([P, SCH], f32, tag="psmm")
                    nc.tensor.matmul(psb[:], ones[:, :],
                                     bias_row[:, ch * SCH:(ch + 1) * SCH],
                                     start=True, stop=True)
                    nc.vector.tensor_copy(
                        bias_bc[:, ch * SCH:(ch + 1) * SCH], psb[:])

                # fp16 context for the composition matmuls (host-rounded;
                # loaded after phase A so it doesn't delay the first matmuls)
                nc.sync.dma_start(
                    ctx_h[:],
                    ctxh_d.ap()[b].rearrange("(st sp) d -> sp st d", sp=P))

                # per q-tile: scores -> softmax -> attn out -> composition
                qTh_r = qTh_d.ap()[b].rearrange("(et ep) q -> ep et q", ep=P)
                qTl_r = qTl_d.ap()[b].rearrange("(et ep) q -> ep et q", ep=P)
                for qt in range(QT):
                    qth = qtp.tile([P, ET, P], f16, tag="qTh")
                    qtl = qtp.tile([P, ET, P], f16, tag="qTl")
                    nc.sync.dma_start(qth[:], qTh_r[:, :, qt * P:(qt + 1) * P])
                    nc.sync.dma_start(qtl[:], qTl_r[:, :, qt * P:(qt + 1) * P])

                    scores = scoresp.tile([P, S], f32, tag="scores")
                    for ch in range(S // SCH):
                        ps = ps_mm.tile([P, SCH], f32, tag="psmm")
                        for e in range(ET):
                            cs = (slice(None), e, slice(ch * SCH, (ch + 1) * SCH))
                            nc.tensor.matmul(ps[:], qth[:, e, :], cwh[cs],
                                             start=(e == 0), stop=False)
                            nc.tensor.matmul(ps[:], qth[:, e, :], cwl[cs],
                                             start=False, stop=False)
                            nc.tensor.matmul(ps[:], qtl[:, e, :], cwh[cs],
                                             start=False, stop=(e == ET - 1))
                        # psum -> sbuf with the mask bias fused in
                        chs = slice(ch * SCH, (ch + 1) * SCH)
                        nc.vector.scalar_tensor_tensor(
                            scores[:, chs], ps[:], 1.0, bias_bc[:, chs],
                            op0=mybir.AluOpType.mult, op1=mybir.AluOpType.add)

                    negmax = smallp.tile([P, 1], f32, tag="negmax")
                    nc.vector.reduce_max(negmax[:], scores[:], axis=AX.X,
                                         negate=True)
                    attn = attnp.tile([P, S], f32, tag="attn")
                    rowsum = smallp.tile([P, 1], f32, tag="rowsum")
                    nc.scalar.activation(attn[:], scores[:], AF.Exp,
                                         bias=negmax[:, :], scale=1.0,
                                         accum_out=rowsum[:])
                    rinv = smallp.tile([P, 1], f32, tag="rinv")
                    nc.vector.reciprocal(rinv[:], rowsum[:])
                    nc.vector.tensor_scalar_mul(attn[:], attn[:], rinv[:, :])
                    nc.sync.dma_start(
                        attn_d.ap()[qt * P:(qt + 1) * P, b, :], attn[:])
                    attn_h = attnhp.tile([P, S], f16, tag="attnh")
                    nc.vector.tensor_copy(attn_h[:], attn[:])

                    pc0 = ps_cp.tile([P, SCH], f32, tag="pscp")
                    pc1 = ps_cp.tile([P, SCH], f32, tag="pscp")
                    pcs = (pc0, pc1)
                    for st in range(ST):
                        pt = ps_tr.tile([P, P], f16, tag="pstr")
                        nc.tensor.transpose(
                            pt[:], attn_h[:, st * P:(st + 1) * P], id_h[:])
                        aT = attnTp.tile([P, P], f16, tag="aT")
                        nc.vector.tensor_copy(aT[:], pt[:])
                        for dch in range(2):
                            nc.tensor.matmul(
                                pcs[dch][:], aT[:],
                                ctx_h[:, st, dch * SCH:(dch + 1) * SCH],
                                start=(st == 0), stop=(st == ST - 1))
                    for dch in range(2):
                        for h in range(2):
                            csb = csbp.tile([P, 256], f32, tag="csb")
                            nc.vector.tensor_copy(
                                csb[:], pcs[dch][:, h * 256:(h + 1) * 256])
                            nc.sync.dma_start(
                                comp_d.ap()[qt * P:(qt + 1) * P, b,
                                            dch * SCH + h * 256:
                                            dch * SCH + (h + 1) * 256],
                                csb[:])

    nc.finalize()
    return nc


def _get_program():
    global _PROG
    if _PROG is None:
        _PROG = _build_program()
    return _PROG


def _split16(x):
    hi = x.astype(np.float16)
    lo = (x - hi.astype(np.float32)).astype(np.float16)
    return hi, lo


def kernel(context, query, W, context_mask):
    global LAST_RESULTS
    from concourse.bass_utils import run_bass_kernel_spmd

    context = np.ascontiguousarray(np.asarray(context), dtype=np.float32)
    query = np.asarray(query, dtype=np.float32)
    W = np.ascontiguousarray(np.asarray(W), dtype=np.float32)
    context_mask = np.asarray(context_mask)

    bias = np.where(context_mask, MASK_BIAS, np.float16(0.0)).astype(np.float16)
    wh, wl = _split16(W)

    in_maps = []
    for c in range(NCORES):
        sl = slice(c * BLOC, (c + 1) * BLOC)
        ctx_sh = context[sl]                                   # [BLOC, S, D]
        ctxT_sh = np.ascontiguousarray(ctx_sh.transpose(0, 2, 1))
        qT_sh = np.ascontiguousarray(
            query[:, sl, :].transpose(1, 2, 0))                # [BLOC, E, Q]
        cth, ctl = _split16(ctxT_sh)
        qth, qtl = _split16(qT_sh)
        in_maps.append({
            "ctxh": ctx_sh.astype(np.float16),
            "ctxTh": cth,
            "ctxTl": ctl,
            "qTh": qth,
            "qTl": qtl,
            "wh": wh,
            "wl": wl,
            "biasr": np.ascontiguousarray(bias[sl]),
        })

    nc = _get_program()
    res = run_bass_kernel_spmd(nc, in_maps, core_ids=list(range(NCORES)))
    LAST_RESULTS = res

    attn_out = np.concatenate(
        [res.results[c]["attn_out"] for c in range(NCORES)], axis=1)
    composition = np.concatenate(
        [res.results[c]["comp_out"] for c in range(NCORES)], axis=1)

    # rows whose whole context is masked get attention (and composition) zeroed
    all_masked = context_mask.all(axis=1)
    if all_masked.any():
        attn_out[:, all_masked, :] = 0.0
        composition[:, all_masked, :] = 0.0

    return attn_out, composition


# revision 17
# speedup vs baseline: 1.0002x; 1.0002x over previous
"""BiLinearAttention Trainium2 kernel (8-core data-parallel over batch).

reference math (per batch b):
    cw      = context[b] @ W                        # [S, E]
    scores  = cw @ query[:, b, :].T  (as [Q, S])
    scores  = where(mask[b], -1e30, scores)
    attn    = softmax(scores, axis=-1)              # [Q, S]
    comp    = attn @ context[b]                     # [Q, D]
outputs: attn_out [Q, B, S], composition [Q, B, D]

Device mapping (per core, 2 batches):
  The two logit matmuls (context@W and cw@q^T) need ~fp32 precision: the
  softmax logits have std ~1024, so low-precision matmuls perturb the
  attention weights catastrophically on near-tie rows. Native fp32 matmul
  runs at 4 cycles/row on the PE. Instead each fp32 operand x is split as
  x = hi + lo (hi = fp16(x), lo = fp16(x - hi), together ~23 mantissa
  bits) and each logical matmul becomes hi*hi + hi*lo + lo*hi at
  1 cycle/row each: 3/4 the PE time of native fp32 with ~fp32 accuracy.
  W/ctxT/qT are split on the host; cwT is split on-device out of PSUM.

  phase A : cwT[e, s] += W[d, e]^T ctxT[d, s]      (fp16 x3)
  scores  : 24 accumulating fp16 matmuls per [128,512] psum chunk
            (lhsT=qT[e-tile, q-tile], rhs=cwT[e-tile, s-chunk]); the mask
            bias is fused into the psum->SBUF copy as a DVE
            scalar_tensor_tensor add against a once-per-batch broadcast
            tile (built by a ones[1,128] (x) bias_row matmul). bias =
            -60000 stands in for the reference's -1e30: fp16-safe, and
            exp(masked - max) underflows to exactly 0 either way.
  softmax : DVE max (negated) -> ACT exp(x - max) with accumulated row
            sum -> DVE reciprocal -> DVE in-place scale (all fp32; the
            fp32 attn tile is DMA'd out directly)
  comp    : attn is also rounded to fp16; PE transposes its 128x128
            tiles (fp16, 1 cyc/row) and 16 fp16 matmuls vs host-rounded
            fp16 context accumulate each [q,d] block. fp16 here is
            harmless: attn is near-one-hot in [0,1] and context is O(1),
            so composition carries ~1e-3 relative error.
"""

import sys

if "/opt/trn_rl_repo" not in sys.path:
    sys.path.insert(0, "/opt/trn_rl_repo")

import numpy as np

B, S, Q, D, E = 16, 2048, 2048, 1024, 1024
NCORES = 8
BLOC = B // NCORES  # batches per core
P = 128
SA = 512   # phase-A s-chunk width
SCH = 512  # scores / comp free-dim chunk width
MASK_BIAS = np.float16(-60000.0)

LAST_RESULTS = None  # BassKernelResults of the most recent run (for test harness)

_PROG = None


def _build_program():
    from concourse import bacc, mybir, tile
    from concourse.masks import make_identity

    f32 = mybir.dt.float32
    f16 = mybir.dt.float16
    AX = mybir.AxisListType
    AF = mybir.ActivationFunctionType

    nc = bacc.Bacc("TRN2", target_bir_lowering=False, debug=False,
                   num_devices=NCORES)
    ctxh_d = nc.dram_tensor("ctxh", [BLOC, S, D], f16, kind="ExternalInput")
    ctxTh_d = nc.dram_tensor("ctxTh", [BLOC, D, S], f16, kind="ExternalInput")
    ctxTl_d = nc.dram_tensor("ctxTl", [BLOC, D, S], f16, kind="ExternalInput")
    qTh_d = nc.dram_tensor("qTh", [BLOC, E, Q], f16, kind="ExternalInput")
    qTl_d = nc.dram_tensor("qTl", [BLOC, E, Q], f16, kind="ExternalInput")
    wh_d = nc.dram_tensor("wh", [D, E], f16, kind="ExternalInput")
    wl_d = nc.dram_tensor("wl", [D, E], f16, kind="ExternalInput")
    bias_d = nc.dram_tensor("biasr", [BLOC, S], f16, kind="ExternalInput")
    attn_d = nc.dram_tensor("attn_out", [Q, BLOC, S], f32, kind="ExternalOutput")
    comp_d = nc.dram_tensor("comp_out", [Q, BLOC, D], f32, kind="ExternalOutput")

    DT, ET, QT, ST = D // P, E // P, Q // P, S // P

    with tile.TileContext(nc) as tc:
        with (
            tc.tile_pool(name="const", bufs=1) as constp,
            tc.tile_pool(name="resident", bufs=1) as resp,
            tc.tile_pool(name="qtp", bufs=2) as qtp,
            tc.tile_pool(name="scoresp", bufs=1) as scoresp,
            tc.tile_pool(name="attnp", bufs=1) as attnp,
            tc.tile_pool(name="attnhp", bufs=1) as attnhp,
            tc.tile_pool(name="attnTp", bufs=4) as attnTp,
            tc.tile_pool(name="ctxT4p", bufs=2) as ctxT4p,
            tc.tile_pool(name="stgp", bufs=1) as stgp,
            tc.tile_pool(name="csbp", bufs=1) as csbp,
            tc.tile_pool(name="smallp", bufs=2) as smallp,
            tc.tile_pool(name="ps_mm", bufs=3, space="PSUM") as ps_mm,
            tc.tile_pool(name="ps_tr", bufs=3, space="PSUM") as ps_tr,
            tc.tile_pool(name="ps_cp", bufs=2, space="PSUM") as ps_cp,
        ):
            wh = constp.tile([P, DT, E], f16)
            wl = constp.tile([P, DT, E], f16)
            wh_r = wh_d.ap().rearrange("(dt dp) e -> dp dt e", dp=P)
            wl_r = wl_d.ap().rearrange("(dt dp) e -> dp dt e", dp=P)
            # chunked so the first phase-A group isn't gated on all 4 MiB
            for ec in range(ET):
                es = slice(ec * P, (ec + 1) * P)
                nc.sync.dma_start(wh[:, :, es], wh_r[:, :, es])
                nc.sync.dma_start(wl[:, :, es], wl_r[:, :, es])
            ones = constp.tile([1, P], f16)
            nc.vector.memset(ones[:], 1.0)
            id_h = constp.tile([P, P], f16)
            id_f32 = stgp.tile([P, P], f32, tag="stg")
            make_identity(nc, id_f32[:])
            nc.vector.tensor_copy(id_h[:], id_f32[:])

            for b in range(BLOC):
                bias_row = resp.tile([1, S], f16, tag="bias_row")
                nc.sync.dma_start(bias_row[:], bias_d.ap()[b:b + 1, :])
                cwh = resp.tile([P, ET, S], f16, tag="cwh")
                cwl = resp.tile([P, ET, S], f16, tag="cwl")
                ctx_h = resp.tile([P, ST, D], f16, tag="ctxh")

                # phase A: cw[e, s] = sum_d W[d, e] * ctxT[d, s], split hi/lo
                ctxTh_r = ctxTh_d.ap()[b].rearrange("(dt dp) s -> dp dt s", dp=P)
                ctxTl_r = ctxTl_d.ap()[b].rearrange("(dt dp) s -> dp dt s", dp=P)
                # first chunk kept small so the opening matmul group
                # gates on ~1 MiB of DMA instead of 4 MiB
                ACH = [(0, 128), (128, 384)] + [
                    (s0, SA) for s0 in range(SA, S, SA)]
                for s0, w in ACH:
                    cth = ctxT4p.tile([P, DT, w], f16, tag="cth")
                    ctl = ctxT4p.tile([P, DT, w], f16, tag="ctl")
                    nc.sync.dma_start(cth[:], ctxTh_r[:, :, s0:s0 + w])
                    nc.sync.dma_start(ctl[:], ctxTl_r[:, :, s0:s0 + w])
                    for e in range(ET):
                        ps = ps_mm.tile([P, SCH], f32, tag="psmm")
                        # hi*hi and lo*hi first: they gate only on cth,
                        # so the ctl DMA overlaps the first 16 matmuls
                        for d in range(DT):
                            we = (slice(None), d, slice(e * P, (e + 1) * P))
                            nc.tensor.matmul(ps[:, :w], wh[we], cth[:, d, :],
                                             start=(d == 0), stop=False)
                            nc.tensor.matmul(ps[:, :w], wl[we], cth[:, d, :],
                                             start=False, stop=False)
                        for d in range(DT):
                            we = (slice(None), d, slice(e * P, (e + 1) * P))
                            nc.tensor.matmul(ps[:, :w], wh[we], ctl[:, d, :],
                                             start=False, stop=(d == DT - 1))
                        sl = (slice(None), e, slice(s0, s0 + w))
                        nc.vector.tensor_copy(cwh[sl], ps[:, :w])
                        nc.vector.tensor_sub(cwl[sl], ps[:, :w], cwh[sl])

                # mask-bias broadcast tile, built after phase A so its
                # (bufs=1) slot swap never stalls the batch boundary
                bias_bc = resp.tile([P, S], f32, tag="bias_bc")
                for ch in range(S // SCH):
                    psb = ps_mm.tile([P, SCH], f32, tag="psmm")
                    nc.tensor.matmul(psb[:], ones[:, :],
                                     bias_row[:, ch * SCH:(ch + 1) * SCH],
                                     start=True, stop=True)
                    nc.vector.tensor_copy(
                        bias_bc[:, ch * SCH:(ch + 1) * SCH], psb[:])

                # fp16 context for the composition matmuls (host-rounded;
                # loaded after phase A so it doesn't delay the first matmuls)
                nc.sync.dma_start(
                    ctx_h[:],
                    ctxh_d.ap()[b].rearrange("(st sp) d -> sp st d", sp=P))

                # per q-tile: scores -> softmax -> attn out -> composition
                qTh_r = qTh_d.ap()[b].rearrange("(et ep) q -> ep et q", ep=P)
                qTl_r = qTl_d.ap()[b].rearrange("(et ep) q -> ep et q", ep=P)
                for qt in range(QT):
                    qth = qtp.tile([P, ET, P], f16, tag="qTh")
                    qtl = qtp.tile([P, ET, P], f16, tag="qTl")
                    nc.sync.dma_start(qth[:], qTh_r[:, :, qt * P:(qt + 1) * P])
                    nc.sync.dma_start(qtl[:], qTl_r[:, :, qt * P:(qt + 1) * P])

                    scores = scoresp.tile([P, S], f32, tag="scores")
                    for ch in range(S // SCH):
                        ps = ps_mm.tile([P, SCH], f32, tag="psmm")
                        for e in range(ET):
                            cs = (slice(None), e, slice(ch * SCH, (ch + 1) * SCH))
                            nc.tensor.matmul(ps[:], qth[:, e, :], cwh[cs],
                                             start=(e == 0), stop=False)
                            nc.tensor.matmul(ps[:], qth[:, e, :], cwl[cs],
                                             start=False, stop=False)
                            nc.tensor.matmul(ps[:], qtl[:, e, :], cwh[cs],
                                             start=False, stop=(e == ET - 1))
                        # psum -> sbuf with the mask bias fused in
                        chs = slice(ch * SCH, (ch + 1) * SCH)
                        nc.vector.scalar_tensor_tensor(
                            scores[:, chs], ps[:], 1.0, bias_bc[:, chs],
                            op0=mybir.AluOpType.mult, op1=mybir.AluOpType.add)

                    negmax = smallp.tile([P, 1], f32, tag="negmax")
                    nc.vector.reduce_max(negmax[:], scores[:], axis=AX.X,
                                         negate=True)
                    attn = attnp.tile([P, S], f32, tag="attn")
                    rowsum = smallp.tile([P, 1], f32, tag="rowsum")
                    nc.scalar.activation(attn[:], scores[:], AF.Exp,
                                         bias=negmax[:, :], scale=1.0,
                                         accum_out=rowsum[:])
                    rinv = smallp.tile([P, 1], f32, tag="rinv")
                    nc.vector.reciprocal(rinv[:], rowsum[:])
                    nc.vector.tensor_scalar_mul(attn[:], attn[:], rinv[:, :])
                    nc.sync.dma_start(
                        attn_d.ap()[qt * P:(qt + 1) * P, b, :], attn[:])
                    attn_h = attnhp.tile([P, S], f16, tag="attnh")
                    nc.vector.tensor_copy(attn_h[:], attn[:])

                    pc0 = ps_cp.tile([P, SCH], f32, tag="pscp")
                    pc1 = ps_cp.tile([P, SCH], f32, tag="pscp")
                    pcs = (pc0, pc1)
                    for st in range(ST):
                        pt = ps_tr.tile([P, P], f16, tag="pstr")
                        nc.tensor.transpose(
                            pt[:], attn_h[:, st * P:(st + 1) * P], id_h[:])
                        aT = attnTp.tile([P, P], f16, tag="aT")
                        nc.vector.tensor_copy(aT[:], pt[:])
                        for dch in range(2):
                            nc.tensor.matmul(
                                pcs[dch][:], aT[:],
                                ctx_h[:, st, dch * SCH:(dch + 1) * SCH],
                                start=(st == 0), stop=(st == ST - 1))
                    for dch in range(2):
                        for h in range(2):
                            csb = csbp.tile([P, 256], f32, tag="csb")
                            nc.vector.tensor_copy(
                                csb[:], pcs[dch][:, h * 256:(h + 1) * 256])
                            nc.sync.dma_start(
                                comp_d.ap()[qt * P:(qt + 1) * P, b,
                                            dch * SCH + h * 256:
                                            dch * SCH + (h + 1) * 256],
                                csb[:])

    nc.finalize()
    return nc


def _get_program():
    global _PROG
    if _PROG is None:
        _PROG = _build_program()
    return _PROG


def _split16(x):
    hi = x.astype(np.float16)
    lo = (x - hi.astype(np.float32)).astype(np.float16)
    return hi, lo


def kernel(context, query, W, context_mask):
    global LAST_RESULTS
    from concourse.bass_utils import run_bass_kernel_spmd

    context = np.ascontiguousarray(np.asarray(context), dtype=np.float32)
    query = np.asarray(query, dtype=np.float32)
    W = np.ascontiguousarray(np.asarray(W), dtype=np.float32)
    context_mask = np.asarray(context_mask)

    bias = np.where(context_mask, MASK_BIAS, np.float16(0.0)).astype(np.float16)
    wh, wl = _split16(W)

    in_maps = []
    for c in range(NCORES):
        sl = slice(c * BLOC, (c + 1) * BLOC)
        ctx_sh = context[sl]                                   # [BLOC, S, D]
        ctxT_sh = np.ascontiguousarray(ctx_sh.transpose(0, 2, 1))
        qT_sh = np.ascontiguousarray(
            query[:, sl, :].transpose(1, 2, 0))                # [BLOC, E, Q]
        cth, ctl = _split16(ctxT_sh)
        qth, qtl = _split16(qT_sh)
        in_maps.append({
            "ctxh": ctx_sh.astype(np.float16),
            "ctxTh": cth,
            "ctxTl": ctl,
            "qTh": qth,
            "qTl": qtl,
            "wh": wh,
            "wl": wl,
            "biasr": np.ascontiguousarray(bias[sl]),
        })

    nc = _get_program()
    res = run_bass_kernel_spmd(nc, in_maps, core_ids=list(range(NCORES)))
    LAST_RESULTS = res

    attn_out = np.concatenate(
        [res.results[c]["attn_out"] for c in range(NCORES)], axis=1)
    composition = np.concatenate(
        [res.results[c]["comp_out"] for c in range(NCORES)], axis=1)

    # rows whose whole context is masked get attention (and composition) zeroed
    all_masked = context_mask.all(axis=1)
    if all_masked.any():
        attn_out[:, all_masked, :] = 0.0
        composition[:, all_masked, :] = 0.0

    return attn_out, composition


# revision 18
# speedup vs baseline: 1.0042x; 1.0040x over previous
"""BiLinearAttention Trainium2 kernel (8-core data-parallel over batch).

reference math (per batch b):
    cw      = context[b] @ W                        # [S, E]
    scores  = cw @ query[:, b, :].T  (as [Q, S])
    scores  = where(mask[b], -1e30, scores)
    attn    = softmax(scores, axis=-1)              # [Q, S]
    comp    = attn @ context[b]                     # [Q, D]
outputs: attn_out [Q, B, S], composition [Q, B, D]

Device mapping (per core, 2 batches):
  The two logit matmuls (context@W and cw@q^T) need ~fp32 precision: the
  softmax logits have std ~1024, so low-precision matmuls perturb the
  attention weights catastrophically on near-tie rows. Native fp32 matmul
  runs at 4 cycles/row on the PE. Instead each fp32 operand x is split as
  x = hi + lo (hi = fp16(x), lo = fp16(x - hi), together ~23 mantissa
  bits) and each logical matmul becomes hi*hi + hi*lo + lo*hi at
  1 cycle/row each: 3/4 the PE time of native fp32 with ~fp32 accuracy.
  W/ctxT/qT are split on the host; cwT is split on-device out of PSUM.

  phase A : cwT[e, s] += W[d, e]^T ctxT[d, s]      (fp16 x3)
  scores  : 24 accumulating fp16 matmuls per [128,512] psum chunk
            (lhsT=qT[e-tile, q-tile], rhs=cwT[e-tile, s-chunk]); the mask
            bias is fused into the psum->SBUF copy as a DVE
            scalar_tensor_tensor add against a once-per-batch broadcast
            tile (built by a ones[1,128] (x) bias_row matmul). bias =
            -60000 stands in for the reference's -1e30: fp16-safe, and
            exp(masked - max) underflows to exactly 0 either way.
  softmax : DVE max (negated) -> ACT exp(x - max) with accumulated row
            sum -> DVE reciprocal -> DVE in-place scale (all fp32; the
            fp32 attn tile is DMA'd out directly)
  comp    : attn is also rounded to fp16; PE transposes its 128x128
            tiles (fp16, 1 cyc/row) and 16 fp16 matmuls vs host-rounded
            fp16 context accumulate each [q,d] block. fp16 here is
            harmless: attn is near-one-hot in [0,1] and context is O(1),
            so composition carries ~1e-3 relative error.
"""

import sys

if "/opt/trn_rl_repo" not in sys.path:
    sys.path.insert(0, "/opt/trn_rl_repo")

import numpy as np

B, S, Q, D, E = 16, 2048, 2048, 1024, 1024
NCORES = 8
BLOC = B // NCORES  # batches per core
P = 128
SA = 512   # phase-A s-chunk width
SCH = 512  # scores / comp free-dim chunk width
MASK_BIAS = np.float16(-60000.0)

LAST_RESULTS = None  # BassKernelResults of the most recent run (for test harness)

_PROG = None


def _build_program():
    from concourse import bacc, mybir, tile
    from concourse.masks import make_identity

    f32 = mybir.dt.float32
    f16 = mybir.dt.float16
    AX = mybir.AxisListType
    AF = mybir.ActivationFunctionType

    nc = bacc.Bacc("TRN2", target_bir_lowering=False, debug=False,
                   num_devices=NCORES)
    ctxh_d = nc.dram_tensor("ctxh", [BLOC, S, D], f16, kind="ExternalInput")
    ctxTh_d = nc.dram_tensor("ctxTh", [BLOC, D, S], f16, kind="ExternalInput")
    ctxTl_d = nc.dram_tensor("ctxTl", [BLOC, D, S], f16, kind="ExternalInput")
    qTh_d = nc.dram_tensor("qTh", [BLOC, E, Q], f16, kind="ExternalInput")
    qTl_d = nc.dram_tensor("qTl", [BLOC, E, Q], f16, kind="ExternalInput")
    wh_d = nc.dram_tensor("wh", [D, E], f16, kind="ExternalInput")
    wl_d = nc.dram_tensor("wl", [D, E], f16, kind="ExternalInput")
    bias_d = nc.dram_tensor("biasr", [BLOC, S], f16, kind="ExternalInput")
    attn_d = nc.dram_tensor("attn_out", [Q, BLOC, S], f32, kind="ExternalOutput")
    comp_d = nc.dram_tensor("comp_out", [Q, BLOC, D], f32, kind="ExternalOutput")

    DT, ET, QT, ST = D // P, E // P, Q // P, S // P

    with tile.TileContext(nc) as tc:
        with (
            tc.tile_pool(name="const", bufs=1) as constp,
            tc.tile_pool(name="resident", bufs=1) as resp,
            tc.tile_pool(name="qtp", bufs=2) as qtp,
            tc.tile_pool(name="scoresp", bufs=1) as scoresp,
            tc.tile_pool(name="attnp", bufs=1) as attnp,
            tc.tile_pool(name="attnhp", bufs=1) as attnhp,
            tc.tile_pool(name="attnTp", bufs=4) as attnTp,
            tc.tile_pool(name="ctxT4p", bufs=2) as ctxT4p,
            tc.tile_pool(name="stgp", bufs=1) as stgp,
            tc.tile_pool(name="csbp", bufs=1) as csbp,
            tc.tile_pool(name="smallp", bufs=2) as smallp,
            tc.tile_pool(name="ps_mm", bufs=3, space="PSUM") as ps_mm,
            tc.tile_pool(name="ps_tr", bufs=3, space="PSUM") as ps_tr,
            tc.tile_pool(name="ps_cp", bufs=2, space="PSUM") as ps_cp,
        ):
            wh = constp.tile([P, DT, E], f16)
            wl = constp.tile([P, DT, E], f16)
            ones = constp.tile([1, P], f16)
            id_h = constp.tile([P, P], f16)

            for b in range(BLOC):
                bias_row = resp.tile([1, S], f16, tag="bias_row")
                nc.sync.dma_start(bias_row[:], bias_d.ap()[b:b + 1, :])
                cwh = resp.tile([P, ET, S], f16, tag="cwh")
                cwl = resp.tile([P, ET, S], f16, tag="cwl")
                ctx_h = resp.tile([P, ST, D], f16, tag="ctxh")

                # phase A: cw[e, s] = sum_d W[d, e] * ctxT[d, s], split hi/lo
                ctxTh_r = ctxTh_d.ap()[b].rearrange("(dt dp) s -> dp dt s", dp=P)
                ctxTl_r = ctxTl_d.ap()[b].rearrange("(dt dp) s -> dp dt s", dp=P)
                for s0, w in [(s0, SA) for s0 in range(0, S, SA)]:
                    cth = ctxT4p.tile([P, DT, w], f16, tag="cth")
                    ctl = ctxT4p.tile([P, DT, w], f16, tag="ctl")
                    nc.sync.dma_start(cth[:], ctxTh_r[:, :, s0:s0 + w])
                    nc.sync.dma_start(ctl[:], ctxTl_r[:, :, s0:s0 + w])
                    if b == 0 and s0 == 0:
                        # W loads issued after the first context chunk so the
                        # opening group's inputs stream concurrently
                        nc.sync.dma_start(
                            wh[:], wh_d.ap().rearrange("(dt dp) e -> dp dt e", dp=P))
                        nc.sync.dma_start(
                            wl[:], wl_d.ap().rearrange("(dt dp) e -> dp dt e", dp=P))
                        nc.vector.memset(ones[:], 1.0)
                        id_f32 = stgp.tile([P, P], f32, tag="stg")
                        make_identity(nc, id_f32[:])
                        nc.vector.tensor_copy(id_h[:], id_f32[:])
                    for e in range(ET):
                        ps = ps_mm.tile([P, SCH], f32, tag="psmm")
                        # hi*hi and lo*hi first: they gate only on cth,
                        # so the ctl DMA overlaps the first 16 matmuls
                        for d in range(DT):
                            we = (slice(None), d, slice(e * P, (e + 1) * P))
                            nc.tensor.matmul(ps[:, :w], wh[we], cth[:, d, :],
                                             start=(d == 0), stop=False)
                            nc.tensor.matmul(ps[:, :w], wl[we], cth[:, d, :],
                                             start=False, stop=False)
                        for d in range(DT):
                            we = (slice(None), d, slice(e * P, (e + 1) * P))
                            nc.tensor.matmul(ps[:, :w], wh[we], ctl[:, d, :],
                                             start=False, stop=(d == DT - 1))
                        sl = (slice(None), e, slice(s0, s0 + w))
                        nc.vector.tensor_copy(cwh[sl], ps[:, :w])
                        nc.vector.tensor_sub(cwl[sl], ps[:, :w], cwh[sl])

                # mask-bias broadcast tile, built after phase A so its
                # (bufs=1) slot swap never stalls the batch boundary
                bias_bc = resp.tile([P, S], f32, tag="bias_bc")
                for ch in range(S // SCH):
                    psb = ps_mm.tile([P, SCH], f32, tag="psmm")
                    nc.tensor.matmul(psb[:], ones[:, :],
                                     bias_row[:, ch * SCH:(ch + 1) * SCH],
                                     start=True, stop=True)
                    nc.vector.tensor_copy(
                        bias_bc[:, ch * SCH:(ch + 1) * SCH], psb[:])

                # fp16 context for the composition matmuls (host-rounded;
                # loaded after phase A so it doesn't delay the first matmuls)
                nc.sync.dma_start(
                    ctx_h[:],
                    ctxh_d.ap()[b].rearrange("(st sp) d -> sp st d", sp=P))

                # per q-tile: scores -> softmax -> attn out -> composition
                qTh_r = qTh_d.ap()[b].rearrange("(et ep) q -> ep et q", ep=P)
                qTl_r = qTl_d.ap()[b].rearrange("(et ep) q -> ep et q", ep=P)
                for qt in range(QT):
                    qth = qtp.tile([P, ET, P], f16, tag="qTh")
                    qtl = qtp.tile([P, ET, P], f16, tag="qTl")
                    nc.sync.dma_start(qth[:], qTh_r[:, :, qt * P:(qt + 1) * P])
                    nc.sync.dma_start(qtl[:], qTl_r[:, :, qt * P:(qt + 1) * P])

                    scores = scoresp.tile([P, S], f32, tag="scores")
                    for ch in range(S // SCH):
                        ps = ps_mm.tile([P, SCH], f32, tag="psmm")
                        for e in range(ET):
                            cs = (slice(None), e, slice(ch * SCH, (ch + 1) * SCH))
                            nc.tensor.matmul(ps[:], qth[:, e, :], cwh[cs],
                                             start=(e == 0), stop=False)
                            nc.tensor.matmul(ps[:], qth[:, e, :], cwl[cs],
                                             start=False, stop=False)
                            nc.tensor.matmul(ps[:], qtl[:, e, :], cwh[cs],
                                             start=False, stop=(e == ET - 1))
                        # psum -> sbuf with the mask bias fused in
                        chs = slice(ch * SCH, (ch + 1) * SCH)
                        nc.vector.scalar_tensor_tensor(
                            scores[:, chs], ps[:], 1.0, bias_bc[:, chs],
                            op0=mybir.AluOpType.mult, op1=mybir.AluOpType.add)

                    negmax = smallp.tile([P, 1], f32, tag="negmax")
                    nc.vector.reduce_max(negmax[:], scores[:], axis=AX.X,
                                         negate=True)
                    attn = attnp.tile([P, S], f32, tag="attn")
                    rowsum = smallp.tile([P, 1], f32, tag="rowsum")
                    nc.scalar.activation(attn[:], scores[:], AF.Exp,
                                         bias=negmax[:, :], scale=1.0,
                                         accum_out=rowsum[:])
                    rinv = smallp.tile([P, 1], f32, tag="rinv")
                    nc.vector.reciprocal(rinv[:], rowsum[:])
                    nc.vector.tensor_scalar_mul(attn[:], attn[:], rinv[:, :])
                    nc.sync.dma_start(
                        attn_d.ap()[qt * P:(qt + 1) * P, b, :], attn[:])
                    attn_h = attnhp.tile([P, S], f16, tag="attnh")
                    nc.vector.tensor_copy(attn_h[:], attn[:])

                    pc0 = ps_cp.tile([P, SCH], f32, tag="pscp")
                    pc1 = ps_cp.tile([P, SCH], f32, tag="pscp")
                    pcs = (pc0, pc1)
                    for st in range(ST):
                        pt = ps_tr.tile([P, P], f16, tag="pstr")
                        nc.tensor.transpose(
                            pt[:], attn_h[:, st * P:(st + 1) * P], id_h[:])
                        aT = attnTp.tile([P, P], f16, tag="aT")
                        nc.vector.tensor_copy(aT[:], pt[:])
                        for dch in range(2):
                            nc.tensor.matmul(
                                pcs[dch][:], aT[:],
                                ctx_h[:, st, dch * SCH:(dch + 1) * SCH],
                                start=(st == 0), stop=(st == ST - 1))
                    for dch in range(2):
                        for h in range(2):
                            csb = csbp.tile([P, 256], f32, tag="csb")
                            nc.vector.tensor_copy(
                                csb[:], pcs[dch][:, h * 256:(h + 1) * 256])
                            nc.sync.dma_start(
                                comp_d.ap()[qt * P:(qt + 1) * P, b,
                                            dch * SCH + h * 256:
                                            dch * SCH + (h + 1) * 256],
                                csb[:])

    nc.finalize()
    return nc


def _get_program():
    global _PROG
    if _PROG is None:
        _PROG = _build_program()
    return _PROG


def _split16(x):
    hi = x.astype(np.float16)
    lo = (x - hi.astype(np.float32)).astype(np.float16)
    return hi, lo


def kernel(context, query, W, context_mask):
    global LAST_RESULTS
    from concourse.bass_utils import run_bass_kernel_spmd

    context = np.ascontiguousarray(np.asarray(context), dtype=np.float32)
    query = np.asarray(query, dtype=np.float32)
    W = np.ascontiguousarray(np.asarray(W), dtype=np.float32)
    context_mask = np.asarray(context_mask)

    bias = np.where(context_mask, MASK_BIAS, np.float16(0.0)).astype(np.float16)
    wh, wl = _split16(W)

    in_maps = []
    for c in range(NCORES):
        sl = slice(c * BLOC, (c + 1) * BLOC)
        ctx_sh = context[sl]                                   # [BLOC, S, D]
        ctxT_sh = np.ascontiguousarray(ctx_sh.transpose(0, 2, 1))
        qT_sh = np.ascontiguousarray(
            query[:, sl, :].transpose(1, 2, 0))                # [BLOC, E, Q]
        cth, ctl = _split16(ctxT_sh)
        qth, qtl = _split16(qT_sh)
        in_maps.append({
            "ctxh": ctx_sh.astype(np.float16),
            "ctxTh": cth,
            "ctxTl": ctl,
            "qTh": qth,
            "qTl": qtl,
            "wh": wh,
            "wl": wl,
            "biasr": np.ascontiguousarray(bias[sl]),
        })

    nc = _get_program()
    res = run_bass_kernel_spmd(nc, in_maps, core_ids=list(range(NCORES)))
    LAST_RESULTS = res

    attn_out = np.concatenate(
        [res.results[c]["attn_out"] for c in range(NCORES)], axis=1)
    composition = np.concatenate(
        [res.results[c]["comp_out"] for c in range(NCORES)], axis=1)

    # rows whose whole context is masked get attention (and composition) zeroed
    all_masked = context_mask.all(axis=1)
    if all_masked.any():
        attn_out[:, all_masked, :] = 0.0
        composition[:, all_masked, :] = 0.0

    return attn_out, composition


# revision 19
# speedup vs baseline: 1.0073x; 1.0031x over previous
"""BiLinearAttention Trainium2 kernel (8-core data-parallel over batch).

reference math (per batch b):
    cw      = context[b] @ W                        # [S, E]
    scores  = cw @ query[:, b, :].T  (as [Q, S])
    scores  = where(mask[b], -1e30, scores)
    attn    = softmax(scores, axis=-1)              # [Q, S]
    comp    = attn @ context[b]                     # [Q, D]
outputs: attn_out [Q, B, S], composition [Q, B, D]

Device mapping (per core, 2 batches):
  The two logit matmuls (context@W and cw@q^T) need ~fp32 precision: the
  softmax logits have std ~1024, so low-precision matmuls perturb the
  attention weights catastrophically on near-tie rows. Native fp32 matmul
  runs at 4 cycles/row on the PE. Instead each fp32 operand x is split as
  x = hi + lo (hi = fp16(x), lo = fp16(x - hi), together ~23 mantissa
  bits) and each logical matmul becomes hi*hi + hi*lo + lo*hi at
  1 cycle/row each: 3/4 the PE time of native fp32 with ~fp32 accuracy.
  W/ctxT/qT are split on the host; cwT is split on-device out of PSUM.

  phase A : cwT[e, s] += W[d, e]^T ctxT[d, s]      (fp16 x3)
  scores  : 24 accumulating fp16 matmuls per [128,512] psum chunk
            (lhsT=qT[e-tile, q-tile], rhs=cwT[e-tile, s-chunk]); the mask
            bias is fused into the psum->SBUF copy as a DVE
            scalar_tensor_tensor add against a once-per-batch broadcast
            tile (built by a ones[1,128] (x) bias_row matmul). bias =
            -60000 stands in for the reference's -1e30: fp16-safe, and
            exp(masked - max) underflows to exactly 0 either way.
  softmax : DVE max (negated) -> ACT exp(x - max) with accumulated row
            sum -> DVE reciprocal -> DVE in-place scale (all fp32; the
            fp32 attn tile is DMA'd out directly)
  comp    : attn is also rounded to fp16; PE transposes its 128x128
            tiles (fp16, 1 cyc/row) and 16 fp16 matmuls vs host-rounded
            fp16 context accumulate each [q,d] block. fp16 here is
            harmless: attn is near-one-hot in [0,1] and context is O(1),
            so composition carries ~1e-3 relative error.
"""

import sys

if "/opt/trn_rl_repo" not in sys.path:
    sys.path.insert(0, "/opt/trn_rl_repo")

import numpy as np

B, S, Q, D, E = 16, 2048, 2048, 1024, 1024
NCORES = 8
BLOC = B // NCORES  # batches per core
P = 128
SA = 512   # phase-A s-chunk width
SCH = 512  # scores / comp free-dim chunk width
MASK_BIAS = np.float16(-60000.0)

LAST_RESULTS = None  # BassKernelResults of the most recent run (for test harness)

_PROG = None


def _build_program():
    from concourse import bacc, mybir, tile
    from concourse.masks import make_identity

    f32 = mybir.dt.float32
    f16 = mybir.dt.float16
    AX = mybir.AxisListType
    AF = mybir.ActivationFunctionType

    nc = bacc.Bacc("TRN2", target_bir_lowering=False, debug=False,
                   num_devices=NCORES)
    ctxh_d = nc.dram_tensor("ctxh", [BLOC, S, D], f16, kind="ExternalInput")
    ctxTh_d = nc.dram_tensor("ctxTh", [BLOC, D, S], f16, kind="ExternalInput")
    ctxTl_d = nc.dram_tensor("ctxTl", [BLOC, D, S], f16, kind="ExternalInput")
    qTh_d = nc.dram_tensor("qTh", [BLOC, E, Q], f16, kind="ExternalInput")
    qTl_d = nc.dram_tensor("qTl", [BLOC, E, Q], f16, kind="ExternalInput")
    wh_d = nc.dram_tensor("wh", [D, E], f16, kind="ExternalInput")
    wl_d = nc.dram_tensor("wl", [D, E], f16, kind="ExternalInput")
    bias_d = nc.dram_tensor("biasr", [BLOC, S], f16, kind="ExternalInput")
    attn_d = nc.dram_tensor("attn_out", [Q, BLOC, S], f32, kind="ExternalOutput")
    comp_d = nc.dram_tensor("comp_out", [Q, BLOC, D], f32, kind="ExternalOutput")

    DT, ET, QT, ST = D // P, E // P, Q // P, S // P

    with tile.TileContext(nc) as tc:
        with (
            tc.tile_pool(name="const", bufs=1) as constp,
            tc.tile_pool(name="resident", bufs=1) as resp,
            tc.tile_pool(name="qtp", bufs=2) as qtp,
            tc.tile_pool(name="scoresp", bufs=1) as scoresp,
            tc.tile_pool(name="attnp", bufs=1) as attnp,
            tc.tile_pool(name="attnhp", bufs=1) as attnhp,
            tc.tile_pool(name="attnTp", bufs=4) as attnTp,
            tc.tile_pool(name="ctxT4p", bufs=2) as ctxT4p,
            tc.tile_pool(name="stgp", bufs=1) as stgp,
            tc.tile_pool(name="csbp", bufs=1) as csbp,
            tc.tile_pool(name="smallp", bufs=2) as smallp,
            tc.tile_pool(name="ps_mm", bufs=3, space="PSUM") as ps_mm,
            tc.tile_pool(name="ps_tr", bufs=3, space="PSUM") as ps_tr,
            tc.tile_pool(name="ps_cp", bufs=2, space="PSUM") as ps_cp,
        ):
            wh = constp.tile([P, DT, E], f16)
            wl = constp.tile([P, DT, E], f16)
            ones = constp.tile([1, P], f16)
            id_h = constp.tile([P, P], f16)

            for b in range(BLOC):
                bias_row = resp.tile([1, S], f16, tag="bias_row")
                nc.sync.dma_start(bias_row[:], bias_d.ap()[b:b + 1, :])
                cwh = resp.tile([P, ET, S], f16, tag="cwh")
                cwl = resp.tile([P, ET, S], f16, tag="cwl")
                ctx_h = resp.tile([P, ST, D], f16, tag="ctxh")

                # phase A: cw[e, s] = sum_d W[d, e] * ctxT[d, s], split hi/lo
                ctxTh_r = ctxTh_d.ap()[b].rearrange("(dt dp) s -> dp dt s", dp=P)
                ctxTl_r = ctxTl_d.ap()[b].rearrange("(dt dp) s -> dp dt s", dp=P)
                for s0, w in [(s0, SA) for s0 in range(0, S, SA)]:
                    cth = ctxT4p.tile([P, DT, w], f16, tag="cth")
                    ctl = ctxT4p.tile([P, DT, w], f16, tag="ctl")
                    if b == 0 and s0 == 0:
                        # opening ramp: interleave per-d-tile pieces of the
                        # first context chunk and W so the first matmuls can
                        # start after ~1 MiB of DMA instead of 8 MiB
                        wh_r = wh_d.ap().rearrange("(dt dp) e -> dp dt e", dp=P)
                        wl_r = wl_d.ap().rearrange("(dt dp) e -> dp dt e", dp=P)
                        for d in range(DT):
                            nc.sync.dma_start(
                                cth[:, d, :], ctxTh_r[:, d, s0:s0 + w])
                            nc.sync.dma_start(wh[:, d, :], wh_r[:, d, :])
                            nc.sync.dma_start(
                                ctl[:, d, :], ctxTl_r[:, d, s0:s0 + w])
                            nc.sync.dma_start(wl[:, d, :], wl_r[:, d, :])
                        nc.vector.memset(ones[:], 1.0)
                        id_f32 = stgp.tile([P, P], f32, tag="stg")
                        make_identity(nc, id_f32[:])
                        nc.vector.tensor_copy(id_h[:], id_f32[:])
                    else:
                        nc.sync.dma_start(cth[:], ctxTh_r[:, :, s0:s0 + w])
                        nc.sync.dma_start(ctl[:], ctxTl_r[:, :, s0:s0 + w])
                    for e in range(ET):
                        ps = ps_mm.tile([P, SCH], f32, tag="psmm")
                        # hi*hi and lo*hi first: they gate only on cth,
                        # so the ctl DMA overlaps the first 16 matmuls
                        for d in range(DT):
                            we = (slice(None), d, slice(e * P, (e + 1) * P))
                            nc.tensor.matmul(ps[:, :w], wh[we], cth[:, d, :],
                                             start=(d == 0), stop=False)
                            nc.tensor.matmul(ps[:, :w], wl[we], cth[:, d, :],
                                             start=False, stop=False)
                        for d in range(DT):
                            we = (slice(None), d, slice(e * P, (e + 1) * P))
                            nc.tensor.matmul(ps[:, :w], wh[we], ctl[:, d, :],
                                             start=False, stop=(d == DT - 1))
                        sl = (slice(None), e, slice(s0, s0 + w))
                        nc.vector.tensor_copy(cwh[sl], ps[:, :w])
                        nc.vector.tensor_sub(cwl[sl], ps[:, :w], cwh[sl])

                # mask-bias broadcast tile, built after phase A so its
                # (bufs=1) slot swap never stalls the batch boundary
                bias_bc = resp.tile([P, S], f32, tag="bias_bc")
                for ch in range(S // SCH):
                    psb = ps_mm.tile([P, SCH], f32, tag="psmm")
                    nc.tensor.matmul(psb[:], ones[:, :],
                                     bias_row[:, ch * SCH:(ch + 1) * SCH],
                                     start=True, stop=True)
                    nc.vector.tensor_copy(
                        bias_bc[:, ch * SCH:(ch + 1) * SCH], psb[:])

                # fp16 context for the composition matmuls (host-rounded;
                # loaded after phase A so it doesn't delay the first matmuls)
                nc.sync.dma_start(
                    ctx_h[:],
                    ctxh_d.ap()[b].rearrange("(st sp) d -> sp st d", sp=P))

                # per q-tile: scores -> softmax -> attn out -> composition
                qTh_r = qTh_d.ap()[b].rearrange("(et ep) q -> ep et q", ep=P)
                qTl_r = qTl_d.ap()[b].rearrange("(et ep) q -> ep et q", ep=P)
                for qt in range(QT):
                    qth = qtp.tile([P, ET, P], f16, tag="qTh")
                    qtl = qtp.tile([P, ET, P], f16, tag="qTl")
                    nc.sync.dma_start(qth[:], qTh_r[:, :, qt * P:(qt + 1) * P])
                    nc.sync.dma_start(qtl[:], qTl_r[:, :, qt * P:(qt + 1) * P])

                    scores = scoresp.tile([P, S], f32, tag="scores")
                    for ch in range(S // SCH):
                        ps = ps_mm.tile([P, SCH], f32, tag="psmm")
                        for e in range(ET):
                            cs = (slice(None), e, slice(ch * SCH, (ch + 1) * SCH))
                            nc.tensor.matmul(ps[:], qth[:, e, :], cwh[cs],
                                             start=(e == 0), stop=False)
                            nc.tensor.matmul(ps[:], qth[:, e, :], cwl[cs],
                                             start=False, stop=False)
                            nc.tensor.matmul(ps[:], qtl[:, e, :], cwh[cs],
                                             start=False, stop=(e == ET - 1))
                        # psum -> sbuf with the mask bias fused in
                        chs = slice(ch * SCH, (ch + 1) * SCH)
                        nc.vector.scalar_tensor_tensor(
                            scores[:, chs], ps[:], 1.0, bias_bc[:, chs],
                            op0=mybir.AluOpType.mult, op1=mybir.AluOpType.add)

                    negmax = smallp.tile([P, 1], f32, tag="negmax")
                    nc.vector.reduce_max(negmax[:], scores[:], axis=AX.X,
                                         negate=True)
                    attn = attnp.tile([P, S], f32, tag="attn")
                    rowsum = smallp.tile([P, 1], f32, tag="rowsum")
                    nc.scalar.activation(attn[:], scores[:], AF.Exp,
                                         bias=negmax[:, :], scale=1.0,
                                         accum_out=rowsum[:])
                    rinv = smallp.tile([P, 1], f32, tag="rinv")
                    nc.vector.reciprocal(rinv[:], rowsum[:])
                    nc.vector.tensor_scalar_mul(attn[:], attn[:], rinv[:, :])
                    nc.sync.dma_start(
                        attn_d.ap()[qt * P:(qt + 1) * P, b, :], attn[:])
                    attn_h = attnhp.tile([P, S], f16, tag="attnh")
                    nc.vector.tensor_copy(attn_h[:], attn[:])

                    pc0 = ps_cp.tile([P, SCH], f32, tag="pscp")
                    pc1 = ps_cp.tile([P, SCH], f32, tag="pscp")
                    pcs = (pc0, pc1)
                    for st in range(ST):
                        pt = ps_tr.tile([P, P], f16, tag="pstr")
                        nc.tensor.transpose(
                            pt[:], attn_h[:, st * P:(st + 1) * P], id_h[:])
                        aT = attnTp.tile([P, P], f16, tag="aT")
                        nc.vector.tensor_copy(aT[:], pt[:])
                        for dch in range(2):
                            nc.tensor.matmul(
                                pcs[dch][:], aT[:],
                                ctx_h[:, st, dch * SCH:(dch + 1) * SCH],
                                start=(st == 0), stop=(st == ST - 1))
                    for dch in range(2):
                        for h in range(2):
                            csb = csbp.tile([P, 256], f32, tag="csb")
                            nc.vector.tensor_copy(
                                csb[:], pcs[dch][:, h * 256:(h + 1) * 256])
                            nc.sync.dma_start(
                                comp_d.ap()[qt * P:(qt + 1) * P, b,
                                            dch * SCH + h * 256:
                                            dch * SCH + (h + 1) * 256],
                                csb[:])

    nc.finalize()
    return nc


def _get_program():
    global _PROG
    if _PROG is None:
        _PROG = _build_program()
    return _PROG


def _split16(x):
    hi = x.astype(np.float16)
    lo = (x - hi.astype(np.float32)).astype(np.float16)
    return hi, lo


def kernel(context, query, W, context_mask):
    global LAST_RESULTS
    from concourse.bass_utils import run_bass_kernel_spmd

    context = np.ascontiguousarray(np.asarray(context), dtype=np.float32)
    query = np.asarray(query, dtype=np.float32)
    W = np.ascontiguousarray(np.asarray(W), dtype=np.float32)
    context_mask = np.asarray(context_mask)

    bias = np.where(context_mask, MASK_BIAS, np.float16(0.0)).astype(np.float16)
    wh, wl = _split16(W)

    in_maps = []
    for c in range(NCORES):
        sl = slice(c * BLOC, (c + 1) * BLOC)
        ctx_sh = context[sl]                                   # [BLOC, S, D]
        ctxT_sh = np.ascontiguousarray(ctx_sh.transpose(0, 2, 1))
        qT_sh = np.ascontiguousarray(
            query[:, sl, :].transpose(1, 2, 0))                # [BLOC, E, Q]
        cth, ctl = _split16(ctxT_sh)
        qth, qtl = _split16(qT_sh)
        in_maps.append({
            "ctxh": ctx_sh.astype(np.float16),
            "ctxTh": cth,
            "ctxTl": ctl,
            "qTh": qth,
            "qTl": qtl,
            "wh": wh,
            "wl": wl,
            "biasr": np.ascontiguousarray(bias[sl]),
        })

    nc = _get_program()
    res = run_bass_kernel_spmd(nc, in_maps, core_ids=list(range(NCORES)))
    LAST_RESULTS = res

    attn_out = np.concatenate(
        [res.results[c]["attn_out"] for c in range(NCORES)], axis=1)
    composition = np.concatenate(
        [res.results[c]["comp_out"] for c in range(NCORES)], axis=1)

    # rows whose whole context is masked get attention (and composition) zeroed
    all_masked = context_mask.all(axis=1)
    if all_masked.any():
        attn_out[:, all_masked, :] = 0.0
        composition[:, all_masked, :] = 0.0

    return attn_out, composition
